# revision 12
# baseline (speedup 1.0000x reference)
"""DenoiseLSTM Trainium2 kernel: 8-core SPMD.

Sharding:
- Encoder (bidirectional LSTM, L=64): replicated on all cores, fp32,
  weight-stationary matmuls, transposed state layout [feat_part, batch].
- Attention K/V + per-step single-query attention: sharded by head (8 heads).
- Decoder LSTM + LayerNorm: replicated (transposed layout).
- Output projection [512, 32000]: vocab-sharded (4000/core), SBUF-resident.
- Greedy feedback: local top-1 (max_with_indices), global combine via
  AllGather collective; next-token embedding via indirect DMA.

Host side (inside kernel()): input sharding/transposes, encoder-input token
embedding lookup, output reassembly.
"""

from contextlib import ExitStack

import numpy as np

import concourse.bass as bass
import concourse.tile as tile
from concourse import bacc, mybir
from concourse import bass_utils
from concourse.masks import make_identity

F32 = mybir.dt.float32
I32 = mybir.dt.int32
U32 = mybir.dt.uint32
U8 = mybir.dt.uint8
AF = mybir.ActivationFunctionType
OP = mybir.AluOpType

P = 128
B = 64
L = 64
V = 32000
NCORE = 8
VS = V // NCORE          # 4000
D_EMB = 128
D_DEC = 512
NH = 8
HD = 64
LN_EPS = 1e-5
NBANK = 8
NB = VS // NBANK         # 500 cols per PSUM bank

_CACHE = {}


def _b_mid(ap, n):
    """[P, F] -> [P, n, F] with stride-0 middle dim."""
    lst = [list(x) for x in ap.ap]
    return bass.AP(ap.tensor, ap.offset, [lst[0], [0, n], *lst[1:]])


def _b_part(ap, parts):
    """[1, ...] -> [parts, ...] stride-0 partition broadcast."""
    lst = [list(x) for x in ap.ap]
    return bass.AP(ap.tensor, ap.offset, [[0, parts], *lst[1:]])


def build(T):
    nc = bacc.Bacc("TRN2", target_bir_lowering=False, debug=False,
                   num_devices=NCORE)

    def din(name, shape, dt=F32):
        return nc.dram_tensor(name, shape, dt, kind="ExternalInput").ap()

    d = dict(
        xT_d=din("xT", [2, L, D_EMB, B]),
        encW_d=din("encW", [2, 384, 1024]),
        encB_d=din("encB", [2, 1024]),
        traW_d=din("traW", [512, D_DEC]),
        c0T_d=din("c0T", [D_DEC, B]),
        xe0T_d=din("xe0T", [D_EMB, B]),
        decW_d=din("decW", [D_EMB + D_DEC, 4 * D_DEC]),
        decB_d=din("decB", [4 * D_DEC]),
        wkvT_d=din("wkvT", [D_DEC, 2 * HD]),
        wqT_d=din("wqT", [D_DEC, HD]),
        bq_d=din("bq", [B, HD]),
        woT_d=din("woT", [D_DEC, D_DEC]),
        misc_d=din("misc", [B, 3, D_DEC]),   # bo_eff, ln_g, ln_b (replicated)
        pw_d=din("pw", [D_DEC, VS]),
        pb_d=din("pb", [B, VS]),
        bofs_d=din("bofs", [B, NBANK]),
        tok_d=din("tok", [V, D_EMB]),
        out_d=nc.dram_tensor("logits", [T, B, VS], F32, kind="ExternalOutput").ap(),
        dbg_d=nc.dram_tensor("dbg", [P, 4, B], F32, kind="ExternalOutput").ap(),
    )
    with tile.TileContext(nc) as tc:
        _build_tile(nc, tc, T, d)
    nc.compile()
    return nc


def _build_tile(nc, tc, T, d):
    with ExitStack() as ctx:
        _build_inner(nc, tc, T, d, ctx)


def _build_inner(nc, tc, T, d, ctx):
    const = ctx.enter_context(tc.tile_pool(name="const", bufs=1))
    ident = const.tile([P, P], F32)
    make_identity(nc, ident)

    mainp = ctx.enter_context(tc.tile_pool(name="mainp", bufs=1))
    h0T = mainp.tile([P, 4, B], F32)
    c0T = mainp.tile([P, 4, B], F32)
    xe0T = mainp.tile([P, B], F32)
    nc.sync.dma_start(c0T[:], d["c0T_d"].rearrange("(c p) b -> p c b", p=P))
    nc.sync.dma_start(xe0T[:], d["xe0T_d"])

    kvp = ctx.enter_context(tc.tile_pool(name="kvp", bufs=1))
    k_t = kvp.tile([B, L, HD], F32)
    v_t = kvp.tile([B, HD, L], F32)

    # ================= ENCODER (replicated, both dirs) =================
    with tc.tile_pool(name="encp", bufs=1) as encp:
        hs = [encp.tile([P, 2, L, B], F32, tag=f"hs{dd}", name=f"hs{dd}")
              for dd in range(2)]
        encW = encp.tile([P, 2, 3, 8, P], F32)
        for dd in range(2):
            nc.sync.dma_start(
                encW[:, dd], d["encW_d"][dd].rearrange("(c p) (g q) -> p c g q", p=P, q=P))
        encB = encp.tile([P, 2, 8], F32)
        nc.sync.dma_start(encB[:], d["encB_d"].rearrange("d (g p) -> p d g", p=P))

        with tc.tile_pool(name="encx", bufs=4) as xp, \
             tc.tile_pool(name="encst", bufs=2) as sp, \
             tc.tile_pool(name="enccs", bufs=2) as cs, \
             tc.tile_pool(name="encps", bufs=4, space="PSUM") as pp:
            cprev = [None, None]
            hploc = [None, None]     # (tile_idx t) of previous h
            for t in range(L):
                for dd in range(2):
                    xt = xp.tile([P, B], F32, tag="xt")
                    nc.sync.dma_start(xt[:], d["xT_d"][dd, t])
                    g = pp.tile([P, 8, B], F32, tag="g", space="PSUM")
                    nk = 3 if t > 0 else 1
                    for gc in range(8):
                        for kc in range(nk):
                            rhs = xt[:] if kc == 0 else hs[dd][:, kc - 1, hploc[dd], :]
                            nc.tensor.matmul(g[:, gc, :], encW[:, dd, kc, gc, :], rhs,
                                             start=(kc == 0), stop=(kc == nk - 1))
                    si = sp.tile([P, 2, B], F32, tag="si")
                    sf = sp.tile([P, 2, B], F32, tag="sf")
                    tg = sp.tile([P, 2, B], F32, tag="tg")
                    so = sp.tile([P, 2, B], F32, tag="so")
                    for j, (dst, fn) in enumerate(
                            [(si, AF.Sigmoid), (sf, AF.Sigmoid), (tg, AF.Tanh), (so, AF.Sigmoid)]):
                        for q in range(2):
                            ch = 2 * j + q
                            nc.scalar.activation(dst[:, q, :], g[:, ch, :], fn,
                                                 bias=encB[:, dd, ch:ch + 1])
                    cnew = cs.tile([P, 2, B], F32, tag=f"c{dd}")
                    nc.vector.tensor_tensor(out=cnew[:], in0=si[:], in1=tg[:], op=OP.mult)
                    if t > 0:
                        tmp = sp.tile([P, 2, B], F32, tag="ctmp")
                        nc.vector.tensor_tensor(out=tmp[:], in0=sf[:], in1=cprev[dd][:],
                                                op=OP.mult)
                        nc.vector.tensor_tensor(out=cnew[:], in0=cnew[:], in1=tmp[:],
                                                op=OP.add)
                    tch = sp.tile([P, 2, B], F32, tag="tch")
                    nc.scalar.activation(tch[:], cnew[:], AF.Tanh)
                    tstore = t if dd == 0 else L - 1 - t
                    nc.vector.tensor_tensor(out=hs[dd][:, :, tstore, :], in0=so[:],
                                            in1=tch[:], op=OP.mult)
                    cprev[dd] = cnew
                    hploc[dd] = tstore

        # ---- h0 = tanh([hf|hb] @ transfer_W.T), transposed ----
        with tc.tile_pool(name="h0p", bufs=1) as h0p, \
             tc.tile_pool(name="h0ps", bufs=1, space="PSUM") as h0ps:
            traW = h0p.tile([P, 4, 4, P], F32)
            nc.sync.dma_start(traW[:],
                              d["traW_d"].rearrange("(c p) (o q) -> p c o q", p=P, q=P))
            hcat = [hs[0][:, 0, L - 1, :], hs[0][:, 1, L - 1, :],
                    hs[1][:, 0, 0, :], hs[1][:, 1, 0, :]]
            ps = h0ps.tile([P, 4, B], F32, space="PSUM")
            for oc in range(4):
                for kc in range(4):
                    nc.tensor.matmul(ps[:, oc, :], traW[:, kc, oc, :], hcat[kc],
                                     start=(kc == 0), stop=(kc == 3))
            nc.scalar.activation(h0T[:], ps[:], AF.Tanh)
        nc.sync.dma_start(d["dbg_d"], h0T[:])

        # ---- K/V for this core's head: k[b,l,hd], v[b,hd,l] ----
        with tc.tile_pool(name="kvw", bufs=1) as kvw, \
             tc.tile_pool(name="kvs", bufs=2) as kvs, \
             tc.tile_pool(name="kvps", bufs=2, space="PSUM") as kvps:
            wkvT = kvw.tile([P, 4, 2 * HD], F32)
            nc.sync.dma_start(wkvT[:], d["wkvT_d"].rearrange("(c p) n -> p c n", p=P))
            for ct in range(8):          # 8 l-values x 64 b = 512 cols per tile
                ps = kvps.tile([P, 8, B], F32, tag="kvps", space="PSUM")
                for kc in range(4):
                    rhs = hs[kc // 2][:, kc % 2, ct * 8:(ct + 1) * 8, :] \
                        .rearrange("p l b -> p (l b)")
                    nc.tensor.matmul(ps[:].rearrange("p l b -> p (l b)"),
                                     wkvT[:, kc, :], rhs,
                                     start=(kc == 0), stop=(kc == 3))
                kvsb = kvs.tile([P, 8, B], F32, tag="kvsb")
                nc.vector.tensor_copy(kvsb[:], ps[:])
                for lsub in range(8):
                    l = ct * 8 + lsub
                    pst = kvps.tile([B, P], F32, tag="pst", space="PSUM")
                    nc.tensor.transpose(pst[:], kvsb[:, lsub, :], ident[:P, :P])
                    nc.vector.tensor_copy(k_t[:, l, :], pst[:, 0:HD])
                    nc.vector.tensor_copy(v_t[:, :, l], pst[:, HD:2 * HD])

    # ================= decoder weights =================
    decp = ctx.enter_context(tc.tile_pool(name="decp", bufs=1))
    decW = decp.tile([P, 5, 16, P], F32)
    nc.sync.dma_start(decW[:], d["decW_d"].rearrange("(c p) (g q) -> p c g q", p=P, q=P))
    decB = decp.tile([P, 16], F32)
    nc.sync.dma_start(decB[:], d["decB_d"].rearrange("(g p) -> p g", p=P))
    wqT = decp.tile([P, 4, HD], F32)
    nc.sync.dma_start(wqT[:], d["wqT_d"].rearrange("(c p) n -> p c n", p=P))
    bq = decp.tile([B, HD], F32)
    nc.sync.dma_start(bq[:], d["bq_d"])
    woT = decp.tile([P, 4, D_DEC], F32)
    nc.sync.dma_start(woT[:], d["woT_d"].rearrange("(c p) n -> p c n", p=P))
    misc = decp.tile([B, 3, D_DEC], F32)
    nc.sync.dma_start(misc[:], d["misc_d"])
    bo, lng, lnb = misc[:, 0, :], misc[:, 1, :], misc[:, 2, :]
    pw = decp.tile([P, 4, VS], F32)
    nc.sync.dma_start(pw[:], d["pw_d"].rearrange("(c p) n -> p c n", p=P))
    pb = decp.tile([B, VS], F32)
    nc.sync.dma_start(pb[:], d["pb_d"])
    bofs = decp.tile([B, NBANK], F32)
    nc.sync.dma_start(bofs[:], d["bofs_d"])
    big = decp.tile([B, NCORE], F32)
    nc.vector.memset(big[:], 1e30)
    epst = decp.tile([B, 1], F32)
    nc.vector.memset(epst[:], LN_EPS)

    # ================= DECODER LOOP =================
    st = ctx.enter_context(tc.tile_pool(name="dst", bufs=1))
    stc = ctx.enter_context(tc.tile_pool(name="dstc", bufs=2))   # carried state
    psg = ctx.enter_context(tc.tile_pool(name="psg", bufs=1, space="PSUM"))
    pss = ctx.enter_context(tc.tile_pool(name="pss", bufs=1, space="PSUM"))
    psp = ctx.enter_context(tc.tile_pool(name="psp", bufs=2, space="PSUM"))
    dram = ctx.enter_context(tc.tile_pool(name="dram", bufs=2, space="DRAM"))

    hT, cT, xeT = h0T, c0T, xe0T
    for t in range(T):
        # ---- LSTM cell (gates transposed [2048 -> 16 chunks, B]) ----
        g = psg.tile([P, 16, B], F32, tag="g", space="PSUM")
        for gc in range(16):
            for kc in range(5):
                rhs = xeT[:] if kc == 0 else hT[:, kc - 1, :]
                nc.tensor.matmul(g[:, gc, :], decW[:, kc, gc, :], rhs,
                                 start=(kc == 0), stop=(kc == 4))
        si = st.tile([P, 4, B], F32, tag="si")
        sf = st.tile([P, 4, B], F32, tag="sf")
        tg = st.tile([P, 4, B], F32, tag="tg")
        so = st.tile([P, 4, B], F32, tag="so")
        for j, (dst, fn) in enumerate(
                [(si, AF.Sigmoid), (sf, AF.Sigmoid), (tg, AF.Tanh), (so, AF.Sigmoid)]):
            for q in range(4):
                ch = 4 * j + q
                nc.scalar.activation(dst[:, q, :], g[:, ch, :], fn,
                                     bias=decB[:, ch:ch + 1])
        cnew = stc.tile([P, 4, B], F32, tag="c")
        tmp = st.tile([P, 4, B], F32, tag="ctmp")
        nc.vector.tensor_tensor(out=cnew[:], in0=si[:], in1=tg[:], op=OP.mult)
        nc.vector.tensor_tensor(out=tmp[:], in0=sf[:], in1=cT[:], op=OP.mult)
        nc.vector.tensor_tensor(out=cnew[:], in0=cnew[:], in1=tmp[:], op=OP.add)
        tcn = st.tile([P, 4, B], F32, tag="tcn")
        nc.scalar.activation(tcn[:], cnew[:], AF.Tanh)
        huT = st.tile([P, 4, B], F32, tag="huT")
        nc.vector.tensor_tensor(out=huT[:], in0=so[:], in1=tcn[:], op=OP.mult)
        cT = cnew

        # h_ normal [B, 512]
        hps = pss.tile([B, D_DEC], F32, tag="sm", space="PSUM")
        for c in range(4):
            nc.tensor.transpose(hps[:, c * P:(c + 1) * P], huT[:, c, :], ident[:P, :P])
        hu = st.tile([B, D_DEC], F32, tag="hu")
        nc.vector.tensor_copy(hu[:], hps[:])

        # ---- attention (own head) ----
        qps = pss.tile([B, HD], F32, tag="sm2", space="PSUM")
        for c in range(4):
            nc.tensor.matmul(qps[:], huT[:, c, :], wqT[:, c, :],
                             start=(c == 0), stop=(c == 3))
        q = st.tile([B, HD], F32, tag="q")
        nc.vector.tensor_tensor(out=q[:], in0=qps[:], in1=bq[:], op=OP.add)
        sc = st.tile([B, L], F32, tag="sc")
        scp = st.tile([B, 8, HD], F32, tag="scp")
        for lc in range(0, L, 8):
            nc.vector.tensor_tensor(out=scp[:], in0=k_t[:, lc:lc + 8, :],
                                    in1=_b_mid(q[:], 8), op=OP.mult)
            nc.vector.tensor_reduce(out=sc[:, lc:lc + 8], in_=scp[:],
                                    axis=mybir.AxisListType.X, op=OP.add)
        mx = st.tile([B, 1], F32, tag="mx")
        nc.vector.tensor_reduce(out=mx[:], in_=sc[:], axis=mybir.AxisListType.X,
                                op=OP.max)
        nmx = st.tile([B, 1], F32, tag="nmx")
        nc.scalar.mul(nmx[:], mx[:], -1.0)
        esc = st.tile([B, L], F32, tag="esc")
        ssum = st.tile([B, 1], F32, tag="ssum")
        nc.scalar.activation(esc[:], sc[:], AF.Exp, bias=nmx[:], accum_out=ssum[:])
        rs = st.tile([B, 1], F32, tag="rs")
        nc.vector.reciprocal(rs[:], ssum[:])
        att = st.tile([B, L], F32, tag="att")
        nc.vector.tensor_scalar_mul(att[:], esc[:], rs[:])
        ctx_ = st.tile([B, HD], F32, tag="ctx")
        ctxp = st.tile([B, 8, L], F32, tag="ctxp")
        for hc in range(0, HD, 8):
            nc.vector.tensor_tensor(out=ctxp[:], in0=v_t[:, hc:hc + 8, :],
                                    in1=_b_mid(att[:], 8), op=OP.mult)
            nc.vector.tensor_reduce(out=ctx_[:, hc:hc + 8], in_=ctxp[:],
                                    axis=mybir.AxisListType.X, op=OP.add)

        # ---- all-gather ctx across heads ----
        cbi = dram.tile([B, HD], F32, tag="cbi")
        cbo = dram.tile([NCORE, B, HD], F32, tag="cbo")
        nc.sync.dma_start(cbi[:], ctx_[:])
        nc.gpsimd.collective_compute(
            "AllGather", OP.bypass, replica_groups=[list(range(NCORE))],
            ins=[cbi[:].opt()], outs=[cbo[:].opt()])
        ctxg = st.tile([B, NH, HD], F32, tag="ctxg")
        nc.sync.dma_start(ctxg[:], cbo[:].rearrange("h b x -> b h x"))

        ctps = pss.tile([P, 4, B], F32, tag="sm3", space="PSUM")
        cgf = ctxg[:].rearrange("b h x -> b (h x)")
        for c in range(4):
            nc.tensor.transpose(ctps[:, c, :], cgf[:, c * P:(c + 1) * P], ident[:B, :B])
        ctxT = st.tile([P, 4, B], F32, tag="ctxT")
        nc.vector.tensor_copy(ctxT[:], ctps[:])
        aps = pss.tile([B, D_DEC], F32, tag="sm", space="PSUM")
        for c in range(4):
            nc.tensor.matmul(aps[:], ctxT[:, c, :], woT[:, c, :],
                             start=(c == 0), stop=(c == 3))

        # ---- residual + LayerNorm ----
        y = st.tile([B, D_DEC], F32, tag="y")
        nc.vector.tensor_tensor(out=y[:], in0=hu[:], in1=aps[:], op=OP.add)
        nc.vector.tensor_tensor(out=y[:], in0=y[:], in1=bo, op=OP.add)
        musum = st.tile([B, 1], F32, tag="musum")
        nc.vector.tensor_reduce(out=musum[:], in_=y[:], axis=mybir.AxisListType.X,
                                op=OP.add)
        nmu = st.tile([B, 1], F32, tag="nmu")
        nc.scalar.mul(nmu[:], musum[:], -1.0 / D_DEC)
        scr = st.tile([B, D_DEC], F32, tag="scr")
        var = st.tile([B, 1], F32, tag="var")
        nc.scalar.activation(scr[:], y[:], AF.Square, bias=nmu[:], accum_out=var[:])
        ycen = st.tile([B, D_DEC], F32, tag="ycen")
        nc.scalar.activation(ycen[:], y[:], AF.Identity, bias=nmu[:])
        sd = st.tile([B, 1], F32, tag="sd")
        nc.scalar.activation(sd[:], var[:], AF.Sqrt, scale=1.0 / D_DEC, bias=epst[:])
        rsd = st.tile([B, 1], F32, tag="rsd")
        nc.vector.reciprocal(rsd[:], sd[:])
        nc.vector.tensor_scalar_mul(ycen[:], ycen[:], rsd[:])
        nc.vector.tensor_tensor(out=ycen[:], in0=ycen[:], in1=lng, op=OP.mult)
        nc.vector.tensor_tensor(out=ycen[:], in0=ycen[:], in1=lnb, op=OP.add)

        hTn = stc.tile([P, 4, B], F32, tag="hTn")
        lps = pss.tile([P, 4, B], F32, tag="sm3", space="PSUM")
        for c in range(4):
            nc.tensor.transpose(lps[:, c, :], ycen[:, c * P:(c + 1) * P], ident[:B, :B])
        nc.vector.tensor_copy(hTn[:], lps[:])
        hT = hTn

        # ---- projection: per-bank evict + bias + DMA + argmax ----
        bkv = st.tile([B, NBANK], F32, tag="bkv")
        bki = st.tile([B, NBANK], F32, tag="bki")
        for nb in range(NBANK):
            pps = psp.tile([B, NB], F32, tag="pps", space="PSUM")
            for c in range(4):
                nc.tensor.matmul(pps[:], hTn[:, c, :], pw[:, c, nb * NB:(nb + 1) * NB],
                                 start=(c == 0), stop=(c == 3))
            lgb = st.tile([B, NB], F32, tag="lgb", bufs=2)
            nc.vector.tensor_tensor(out=lgb[:], in0=pps[:],
                                    in1=pb[:, nb * NB:(nb + 1) * NB], op=OP.add)
            nc.sync.dma_start(d["out_d"][t, :, nb * NB:(nb + 1) * NB], lgb[:])
            bv8 = st.tile([B, 8], F32, tag="bv8")
            bi8 = st.tile([B, 8], U32, tag="bi8")
            nc.vector.max_with_indices(bv8[:], bi8[:], lgb[:])
            nc.vector.tensor_copy(bkv[:, nb:nb + 1], bv8[:, 0:1])
            bif = st.tile([B, 1], F32, tag="bif")
            nc.vector.tensor_copy(bif[:], bi8[:, 0:1])
            nc.vector.tensor_tensor(out=bki[:, nb:nb + 1], in0=bif[:],
                                    in1=bofs[:, nb:nb + 1], op=OP.add)
        # local winner across banks
        lwv = st.tile([B, 1], F32, tag="lwv")
        nc.vector.tensor_reduce(out=lwv[:], in_=bkv[:], axis=mybir.AxisListType.X,
                                op=OP.max)
        lmsk = st.tile([B, NBANK], U8, tag="lmsk")
        nc.vector.tensor_scalar(out=lmsk[:], in0=bkv[:], scalar1=lwv[:],
                                scalar2=None, op0=OP.is_equal)
        lcand = st.tile([B, NBANK], F32, tag="lcand")
        nc.vector.select(lcand[:], lmsk[:], bki[:], big[:])
        gidx = st.tile([B, 1], F32, tag="gidx")
        nc.vector.tensor_reduce(out=gidx[:], in_=lcand[:], axis=mybir.AxisListType.X,
                                op=OP.min)
        lv = st.tile([B, 2], F32, tag="lv")
        nc.vector.tensor_copy(lv[:, 0:1], lwv[:])
        nc.vector.tensor_copy(lv[:, 1:2], gidx[:])

        # ---- all-gather (val, idx) + global winner ----
        abi = dram.tile([B, 2], F32, tag="abi")
        abo = dram.tile([NCORE, B, 2], F32, tag="abo")
        nc.sync.dma_start(abi[:], lv[:])
        nc.gpsimd.collective_compute(
            "AllGather", OP.bypass, replica_groups=[list(range(NCORE))],
            ins=[abi[:].opt()], outs=[abo[:].opt()])
        lvg = st.tile([B, NCORE, 2], F32, tag="lvg")
        nc.sync.dma_start(lvg[:], abo[:].rearrange("c b x -> b c x"))

        wv = st.tile([B, 1], F32, tag="wv")
        nc.vector.tensor_reduce(out=wv[:], in_=lvg[:, :, 0], axis=mybir.AxisListType.X,
                                op=OP.max)
        msk = st.tile([B, NCORE], U8, tag="msk")
        nc.vector.tensor_scalar(out=msk[:], in0=lvg[:, :, 0], scalar1=wv[:],
                                scalar2=None, op0=OP.is_equal)
        cand = st.tile([B, NCORE], F32, tag="cand")
        nc.vector.select(cand[:], msk[:], lvg[:, :, 1], big[:])
        widx = st.tile([B, 1], F32, tag="widx")
        nc.vector.tensor_reduce(out=widx[:], in_=cand[:], axis=mybir.AxisListType.X,
                                op=OP.min)

        # ---- next token embedding ----
        widxi = st.tile([B, 1], I32, tag="widxi")
        nc.vector.tensor_copy(widxi[:], widx[:])
        xe = st.tile([B, D_EMB], F32, tag="xe")
        nc.gpsimd.indirect_dma_start(
            out=xe[:], out_offset=None, in_=d["tok_d"],
            in_offset=bass.IndirectOffsetOnAxis(ap=widxi[:, :1], axis=0))
        xps = pss.tile([P, B], F32, tag="sm2", space="PSUM")
        nc.tensor.transpose(xps[:], xe[:], ident[:B, :B])
        xeTn = stc.tile([P, B], F32, tag="xeTn")
        nc.vector.tensor_copy(xeTn[:], xps[:])
        xeT = xeTn


def kernel(**inputs):
    nx = np.asarray(inputs["nx"]).astype(np.int64)
    label = np.asarray(inputs["label"]).astype(np.int64)
    T = int(np.asarray(inputs["max_len"]))
    f32 = lambda k: np.asarray(inputs[k], np.float32)
    start_emb, tok_emb, style_emb = f32("start_emb"), f32("tok_emb"), f32("style_emb")
    proj_W, proj_b = f32("proj_W"), f32("proj_b")

    x = tok_emb[nx]                                   # [B, L, 128]
    xT = np.ascontiguousarray(
        np.stack([x.transpose(1, 2, 0), x[:, ::-1].transpose(1, 2, 0)]))

    def enc_dir(s):
        w = np.concatenate([f32(f"enc_Wih_{s}"), f32(f"enc_Whh_{s}")], axis=1)
        return w.T                                    # [384, 1024]
    encW = np.ascontiguousarray(np.stack([enc_dir("f"), enc_dir("b")]))
    encB = np.stack([f32("enc_b_f"), f32("enc_b_b")])

    traW = np.ascontiguousarray(f32("transfer_W").T)
    c0T = np.ascontiguousarray(style_emb[label].T)
    xe0T = np.ascontiguousarray(np.repeat(start_emb.T, B, axis=1))

    decW = np.ascontiguousarray(
        np.concatenate([f32("dec_Wih"), f32("dec_Whh")], axis=1).T)
    decB = f32("dec_b")

    aw, ab = f32("attn_in_w"), f32("attn_in_b")
    Wq, Wk, Wv = np.split(aw, 3, axis=0)
    bq_, bk_, bv_ = np.split(ab, 3, axis=0)
    scale = np.float32(1.0 / np.sqrt(HD))
    wo, bo_ = f32("attn_out_w"), f32("attn_out_b")
    bo_eff = bo_ + bv_ @ wo.T
    misc = np.repeat(np.stack([bo_eff, f32("ln_g"), f32("ln_b")])[None], B, axis=0)
    misc = np.ascontiguousarray(misc)

    in_maps = []
    for c in range(NCORE):
        hsl = slice(c * HD, (c + 1) * HD)
        vsl = slice(c * VS, (c + 1) * VS)
        in_maps.append(dict(
            xT=xT, encW=encW, encB=encB, traW=traW, c0T=c0T, xe0T=xe0T,
            decW=decW, decB=decB,
            wkvT=np.ascontiguousarray(
                np.concatenate([Wk[hsl], Wv[hsl]], axis=0).T),
            wqT=np.ascontiguousarray((Wq[hsl] * scale).T),
            bq=np.repeat((bq_[hsl] * scale)[None, :], B, axis=0).copy(),
            woT=np.ascontiguousarray(wo.T),
            misc=misc,
            pw=np.ascontiguousarray(proj_W[vsl].T),
            pb=np.ascontiguousarray(np.repeat(proj_b[vsl][None, :], B, axis=0)),
            bofs=np.tile((np.arange(NBANK) * NB + c * VS).astype(np.float32), (B, 1)),
            tok=tok_emb,
        ))

    if T not in _CACHE:
        _CACHE[T] = build(T)
    nc = _CACHE[T]

    global _LAST_IN_MAPS
    _LAST_IN_MAPS = in_maps
    res = bass_utils.run_bass_kernel_spmd(nc, in_maps, core_ids=list(range(NCORE)))
    shards = [res.results[c]["logits"] for c in range(NCORE)]
    full = np.concatenate(shards, axis=2)             # [T, B, V]
    return np.ascontiguousarray(full.transpose(1, 0, 2))
